# revision 14
# baseline (speedup 1.0000x reference)
"""DGAT attention head on 8 trn2 NeuronCores.

Sharding: row-wise over query nodes (core c owns rows [c*R, (c+1)*R)).
Each core receives its slice of adj PRE-TRANSPOSED (j-major layout, a
host-side sharding/layout choice) so that on device:
  - the softmax mask multiply runs j-on-partitions (matches matmul needs)
  - attention @ h contracts j on the PE with no on-device transposes of
    the big [N, N] tensor.

Math (exact for binary adj in {0,1}):
  h   = x @ w
  hz1 = h @ a[:D] = x @ (w @ a[:D]), hz2 likewise
  e   = leaky(C*(hz1[i] + hz2[j]) + D0)           (leaky slope 0.2)
  logits(i,j) = leaky(A*adj+B)*e  masked to -inf where adj==0
  For adj binary: masked logits = L1*e where adj==1, L1 = leaky(A+B).
  softmax row-wise == (exp(L1*e - G) * adj) normalized, any constant G.
  G is a host-derived stability bound (max of L1*leaky(C*(u+v)+D0) over
  extreme combos of u in hz1-range, v in hz2-range).
  out = elu((p @ h) / (p @ 1)) with p = exp(L1*e - G)*adj.

On-device pipeline per 256-row j-megatile (j on partitions, i free):
  DMA 1MB adjT -> ACT Lrelu (scale=C, per-partition bias col) ->
  ACT Exp (scale=L1, bias=-G) -> DVE mask-mul -> PE matmul accumulate
  psum[65, R] with lhsT = [h | 1] per 128-j group.
Tail: psum -> sbuf, PE transposes [65,128]->[128,65], reciprocal of
the sum column, ELU composed as relu(v) + (min(exp(v),1) - 1).
"""

import numpy as np

import concourse.bass as bass
import concourse.bacc as bacc
import concourse.mybir as mybir
from concourse.tile import TileContext
from concourse.bass_utils import run_bass_kernel_spmd
from concourse.masks import make_identity

F32 = mybir.dt.float32
AF = mybir.ActivationFunctionType
OP = mybir.AluOpType

NCORES = 8
SLOPE = 0.2  # leakyrelu negative slope (fixed in the reference)

# set by test harness to capture a profile
TRACE = False
LAST_RESULTS = None
LAST_NC = None


def _leaky(z):
    return z if z >= 0.0 else SLOPE * z


def _build(n, din, dout, rows, C, D0, L1, G):
    """Build the SPMD Bass program (identical on all cores)."""
    assert n % 256 == 0 and rows % 128 == 0 and din % 128 == 0
    ng = n // 128          # j-groups of 128
    nm = n // 256          # j-megatiles of 256 (1MB f32 DMA at rows=1024)
    kc = din // 128        # contraction chunks
    grp = 4                # j-groups per preamble psum bank
    assert ng % grp == 0
    de = dout + 1          # h columns + ones column

    nc = bacc.Bacc("TRN2", target_bir_lowering=False)
    adjt_d = nc.dram_tensor("adjt", [n, rows], F32, kind="ExternalInput")
    xt_d = nc.dram_tensor("xt", [din, n], F32, kind="ExternalInput")
    xto_d = nc.dram_tensor("xt_own", [din, rows], F32, kind="ExternalInput")
    w_d = nc.dram_tensor("w", [din, dout], F32, kind="ExternalInput")
    a_d = nc.dram_tensor("a", [2 * dout, 1], F32, kind="ExternalInput")
    y_d = nc.dram_tensor("y", [rows, dout], F32, kind="ExternalOutput")

    with TileContext(nc) as tc:
        with (
            tc.tile_pool(name="consts", bufs=1) as consts,
            tc.tile_pool(name="adjp", bufs=3) as adjp,
            tc.tile_pool(name="etp", bufs=2) as etp,
            tc.tile_pool(name="qtp", bufs=2) as qtp,
            tc.tile_pool(name="ptp", bufs=2) as ptp,
            tc.tile_pool(name="tailp", bufs=2) as tailp,
        ):
            # identity for tail PE transposes; re-copied on DVE so the
            # transposes depend on a single (DVE) semaphore — transpose-mode
            # Matmult has only one HW wait slot.
            identity0 = consts.tile([128, 128], F32)
            make_identity(nc, identity0)
            identity = consts.tile([128, 128], F32)
            nc.vector.tensor_copy(identity, identity0)

            negG = consts.tile([128, 1], F32)
            nc.vector.memset(negG, -G)
            zcol = consts.tile([128, 1], F32)
            nc.vector.memset(zcol, 0.0)
            ones128 = consts.tile([128, 128], F32)
            nc.vector.memset(ones128, 1.0)

            # a1/a2 broadcast across partitions (partition-step-0 DMA)
            a_ap = a_d[:, :]
            a1bc = consts.tile([128, dout], F32)
            nc.gpsimd.dma_start(
                out=a1bc,
                in_=bass.AP(tensor=a_ap.tensor, offset=0, ap=[[0, 128], [1, dout]]),
            )
            a2bc = consts.tile([128, dout], F32)
            nc.gpsimd.dma_start(
                out=a2bc,
                in_=bass.AP(
                    tensor=a_ap.tensor, offset=dout, ap=[[0, 128], [1, dout]]
                ),
            )

            # wx_k = [w_k | w_k@a1 | w_k@a2], single-writer consolidated
            wx = []
            for k in range(kc):
                wxr = consts.tile([128, dout + 2], F32, name=f"wxr{k}")
                nc.sync.dma_start(
                    out=wxr[:, 0:dout], in_=w_d[k * 128 : (k + 1) * 128, :]
                )
                t1 = consts.tile([128, dout], F32, name=f"wa_t{k}")
                nc.vector.tensor_mul(t1, wxr[:, 0:dout], a1bc)
                nc.vector.reduce_sum(
                    wxr[:, dout : dout + 1], t1, axis=mybir.AxisListType.X
                )
                t2 = consts.tile([128, dout], F32, name=f"wb_t{k}")
                nc.vector.tensor_mul(t2, wxr[:, 0:dout], a2bc)
                nc.vector.reduce_sum(
                    wxr[:, dout + 1 : dout + 2], t2, axis=mybir.AxisListType.X
                )
                wxk = consts.tile([128, dout + 2], F32, name=f"wx{k}")
                nc.vector.tensor_copy(wxk, wxr)
                wx.append(wxk)

            h_ext = consts.tile([128, ng, de], F32)
            nc.vector.memset(h_ext[:, :, dout : dout + 1], 1.0)
            hz1bc = consts.tile([128, rows], F32)
            hz2cols = consts.tile([128, ng], F32)
            bias_cols = consts.tile([128, ng], F32)
            hpT = consts.tile([de, rows], F32)

            with (
                tc.tile_pool(name="xtp", bufs=1) as xtp,
                tc.tile_pool(name="pspre", bufs=2, space="PSUM") as pspre,
                tc.tile_pool(name="pshz", bufs=1, space="PSUM") as pshz,
            ):
                xts = []
                for k in range(kc):
                    xtk = xtp.tile([128, n], F32, name=f"xt{k}")
                    nc.sync.dma_start(
                        out=xtk, in_=xt_d[k * 128 : (k + 1) * 128, :]
                    )
                    xts.append(xtk)
                xtos = []
                for k in range(kc):
                    xtok = xtp.tile([128, rows], F32, name=f"xto{k}")
                    nc.sync.dma_start(
                        out=xtok, in_=xto_d[k * 128 : (k + 1) * 128, :]
                    )
                    xtos.append(xtok)

                # h_ext and hz2 columns: per j-group, [h_j | hz1_j | hz2_j]
                for g0 in range(0, ng, grp):
                    ps = pspre.tile([128, grp, dout + 2], F32, name="ps_h")
                    for gi in range(grp):
                        g = g0 + gi
                        for k in range(kc):
                            nc.tensor.matmul(
                                ps[:, gi, :],
                                xts[k][:, g * 128 : (g + 1) * 128],
                                wx[k],
                                start=(k == 0),
                                stop=(k == kc - 1),
                            )
                    nc.vector.tensor_copy(
                        h_ext[:, g0 : g0 + grp, 0:dout], ps[:, :, 0:dout]
                    )
                    nc.vector.tensor_copy(
                        hz2cols[:, g0 : g0 + grp],
                        ps[:, :, dout + 1 : dout + 2],
                    )

                # hz1 broadcast row-block [128, rows] from own x columns:
                # hz1bc[p, i] = sum_k wa1[k] * x[i, k] for own i
                hz_ps = pshz.tile([128, rows], F32)
                for k in range(kc):
                    wa1bc = consts.tile([128, 128], F32, name=f"wa1bc{k}")
                    nc.vector.tensor_scalar_mul(
                        wa1bc, ones128, wx[k][:, dout : dout + 1]
                    )
                    for n0 in range(0, rows, 512):
                        nw = min(512, rows - n0)
                        nc.tensor.matmul(
                            hz_ps[:, n0 : n0 + nw],
                            wa1bc,
                            xtos[k][:, n0 : n0 + nw],
                            start=(k == 0),
                            stop=(k == kc - 1),
                        )
                nc.vector.tensor_copy(hz1bc, hz_ps)
                nc.vector.tensor_scalar(
                    bias_cols, hz2cols, C, D0, OP.mult, OP.add
                )

            # main loop: stream adjT megatiles (256 j x rows i), j-major
            adjt_r = adjt_d[:, :].rearrange(
                "(m t p) i -> m p t i", t=2, p=128
            )
            with (
                tc.tile_pool(name="psacc", bufs=1, space="PSUM") as psacc,
                tc.tile_pool(name="pstail", bufs=2, space="PSUM") as pstail,
            ):
                acc = psacc.tile([de, rows], F32)
                for m in range(nm):
                    adjt_t = adjp.tile([128, 2 * rows], F32)
                    nc.sync.dma_start(
                        out=adjt_t.rearrange("p (t i) -> p t i", t=2),
                        in_=adjt_r[m],
                    )
                    et = etp.tile([128, 2 * rows], F32)
                    for t in range(2):
                        g = 2 * m + t
                        nc.scalar.activation(
                            et[:, t * rows : (t + 1) * rows],
                            hz1bc,
                            AF.Prelu,
                            bias=bias_cols[:, g : g + 1],
                            scale=C,
                            alpha=SLOPE,
                        )
                    qt = qtp.tile([128, 2 * rows], F32)
                    nc.scalar.activation(
                        qt, et, AF.Exp, bias=negG[:, 0:1], scale=L1
                    )
                    pt = ptp.tile([128, 2 * rows], F32)
                    nc.vector.tensor_mul(pt, qt, adjt_t)
                    for t in range(2):
                        g = 2 * m + t
                        for n0 in range(0, rows, 512):
                            nw = min(512, rows - n0)
                            nc.tensor.matmul(
                                acc[:, n0 : n0 + nw],
                                h_ext[:, g, :],
                                pt[:, t * rows + n0 : t * rows + n0 + nw],
                                start=(g == 0),
                                stop=(g == ng - 1),
                            )

                # tail: normalize + elu, back to i-major
                nc.vector.tensor_copy(hpT, acc)
                for cc in range(rows // 128):
                    tp = pstail.tile([128, de], F32)
                    nc.tensor.transpose(
                        tp,
                        hpT[:, cc * 128 : (cc + 1) * 128],
                        identity[0:de, 0:de],
                    )
                    recip = tailp.tile([128, 1], F32)
                    nc.vector.reciprocal(recip, tp[:, dout : dout + 1])
                    v = tailp.tile([128, dout], F32)
                    nc.vector.tensor_scalar_mul(v, tp[:, 0:dout], recip)
                    r = tailp.tile([128, dout], F32)
                    nc.scalar.activation(r, v, AF.Relu, bias=zcol[:, 0:1])
                    q = tailp.tile([128, dout], F32)
                    nc.scalar.activation(q, v, AF.Exp, bias=zcol[:, 0:1])
                    m2 = tailp.tile([128, dout], F32)
                    nc.vector.tensor_scalar(
                        m2, q, 1.0, -1.0, OP.min, OP.add
                    )
                    ysb = tailp.tile([128, dout], F32)
                    nc.vector.tensor_add(ysb, r, m2)
                    nc.sync.dma_start(
                        out=y_d[cc * 128 : (cc + 1) * 128, :], in_=ysb
                    )
    nc.compile()
    return nc


def _run(x, adj, w, a, a_coeff, b_coeff, c_coeff, d_coeff):
    global LAST_RESULTS, LAST_NC
    n, din = x.shape
    dout = w.shape[1]
    assert adj.shape == (n, n) and a.shape == (2 * dout, 1)
    rows = n // NCORES

    A = float(np.asarray(a_coeff).reshape(-1)[0])
    B = float(np.asarray(b_coeff).reshape(-1)[0])
    C = float(np.asarray(c_coeff).reshape(-1)[0])
    D0 = float(np.asarray(d_coeff).reshape(-1)[0])
    L1 = _leaky(A + B)

    x = np.ascontiguousarray(x, dtype=np.float32)
    adj = np.asarray(adj, dtype=np.float32)
    w = np.ascontiguousarray(w, dtype=np.float32)
    a = np.ascontiguousarray(a, dtype=np.float32)

    # host-side stability shift G >= max logit (cheap: only h extremes)
    h = x @ w
    hz1 = h @ a[:dout, 0]
    hz2 = h @ a[dout:, 0]
    cand = []
    for u in (hz1.min(), hz1.max()):
        for v in (hz2.min(), hz2.max()):
            cand.append(L1 * _leaky(C * (float(u) + float(v)) + D0))
    G = float(max(cand))

    nc = _build(n, din, dout, rows, C, D0, L1, G)
    LAST_NC = nc

    xt = np.ascontiguousarray(x.T)
    in_maps = []
    for c in range(NCORES):
        sl = slice(c * rows, (c + 1) * rows)
        in_maps.append(
            {
                "adjt": np.ascontiguousarray(adj[sl, :].T),
                "xt": xt,
                "xt_own": np.ascontiguousarray(xt[:, sl]),
                "w": w,
                "a": a,
            }
        )

    res = run_bass_kernel_spmd(
        nc, in_maps, core_ids=list(range(NCORES)), trace=TRACE
    )
    LAST_RESULTS = res
    return np.concatenate([r["y"] for r in res.results], axis=0)


def kernel(x, adj, w, a, a_coeff, b_coeff, c_coeff, d_coeff):
    return _run(x, adj, w, a, a_coeff, b_coeff, c_coeff, d_coeff)


# revision 21
# speedup vs baseline: 1.7347x; 1.7347x over previous
"""DGAT attention head on 8 trn2 NeuronCores.

Sharding: row-wise over query nodes (core c owns rows [c*R, (c+1)*R)).
Each core receives its adj slice pre-transposed and mask-encoded
(host-side layout choice): adjt_enc = (adj^T - 1) * BIG in bf16
(exact: adj is binary), so masking becomes an additive logit bias.

Math (exact for binary adj):
  h   = x @ w;  hz1 = x @ (w @ a[:D]);  hz2 = x @ (w @ a[D:])
  z   = C*(hz1[i] + hz2[j]) + D0;  L1 = leaky(A+B)   (leaky slope 0.2)
  row-softmax of masked logits  ==  normalize(exp(L1*leaky(z) - G + BIG*(adj-1)))
  out = elu((p @ h) / (p @ 1))
G is a host-derived bound on max logit (from hz1/hz2 extremes).

Device pipeline per 256-j megatile (j on partitions, i free):
  DMA 512KB bf16 adjt_enc
  -> custom DVE op: u = leakyscaled(Src0 + bias_col) + adjt_enc - G   (1 pass)
  -> ACT Exp: q = exp(u), bf16 out                                    (1 pass)
  -> PE: psum[65, R] += [h|1]^T-group @ q-half (bf16, N=1024)
Tail: psum -> sbuf, PE transposes [65,128]->[128,65], reciprocal of the
sum column, ELU = relu(v) + (min(exp(v),1) - 1).

The leaky-scale trick: for L1>=0, u_leak = select(w>=0, w, 0.2w) with
w = L1*z (positive homogeneity); for L1<0, w = 0.2*L1*z and the false
branch multiplies by 1/0.2.
"""

import numpy as np
import ml_dtypes

import concourse.bass as bass
import concourse.bacc as bacc
import concourse.mybir as mybir
import concourse.dve_ops as dve_ops
from concourse.dve_spec import Spec, Src0, Src1, C0, C1, C2, Zero, select
from concourse.tile import TileContext
from concourse.bass_utils import run_bass_kernel_spmd

F32 = mybir.dt.float32
F16 = mybir.dt.float16
F32R = mybir.dt.float32r
AF = mybir.ActivationFunctionType
OP = mybir.AluOpType

NCORES = 8
SLOPE = 0.2    # leakyrelu negative slope (fixed in the reference)
BIG = 16384.0  # additive mask magnitude (exact in fp16; exp(-BIG) == 0)

TRACE = False
LAST_RESULTS = None
LAST_NC = None


def _leaky(z):
    return z if z >= 0.0 else SLOPE * z


def _register_leaky_mask_op():
    name = "LEAKY_MASK_BIAS_ANT"
    for op in dve_ops.OPS:
        if op.name == name:
            return op
    w = Src0 + C0
    spec = Spec(
        body=select(w >= Zero, w, w * C1) + Src1 + C2,
        reference=lambda in0, in1, s0, s1, imm2: (
            np.where(in0 + s0 >= 0, in0 + s0, (in0 + s0) * s1) + in1 + imm2
        ).astype(np.float32),
    )
    op = dve_ops.DveOp(name, spec, subdim=False, uops_sha={})
    dve_ops.OPS.append(op)
    dve_ops.CUSTOM_DVE_SPECS[name] = spec
    dve_ops._SUB_OPCODE_FOR_NAME[name] = (
        dve_ops._CUSTOM_DVE_ROW_BASE + len(dve_ops.OPS) - 1
    )
    assert dve_ops._SUB_OPCODE_FOR_NAME[name] < 0x20
    for ver in ("v3",):
        try:
            op.compile(ver)
        except ValueError as e:
            msg = str(e)
            key = 'uops_sha["%s"]="' % ver
            i = msg.index(key) + len(key)
            op.uops_sha[ver] = msg[i : msg.index('"', i)]
        dve_ops._COMPILE_CACHE.pop((name, ver), None)
        op.compile(ver)
    return op


def _build(n, din, dout, rows, kpre, s1_slope, G):
    """Build the SPMD Bass program (identical on all cores).

    kpre: scale applied to hz1/hz2 logit halves (= k*C with k = L1 or
    SLOPE*L1); the per-partition bias col is kpre*hz2 + kD (kD folded on
    device); s1_slope: false-branch slope of the select (0.2 or 5.0).
    """
    assert n % 256 == 0 and rows % 128 == 0 and din % 128 == 0
    ng = n // 128
    nm = n // 256
    kc = din // 128
    grp = 4
    assert ng % grp == 0
    de = dout + 1
    lmb = _register_leaky_mask_op()

    nc = bacc.Bacc("TRN2", target_bir_lowering=False)
    adjt_d = nc.dram_tensor("adjt", [n, rows], F16, kind="ExternalInput")
    xt_d = nc.dram_tensor("xt", [din, n], F16, kind="ExternalInput")
    xto_d = nc.dram_tensor("xt_own", [din, rows], F16, kind="ExternalInput")
    w_d = nc.dram_tensor("w", [din, dout], F32, kind="ExternalInput")
    a_d = nc.dram_tensor("a", [2 * dout, 1], F32, kind="ExternalInput")
    kd_d = nc.dram_tensor("kd", [1, 1], F32, kind="ExternalInput")
    y_d = nc.dram_tensor("y", [rows, dout], F32, kind="ExternalOutput")

    with TileContext(nc) as tc:
        with (
            tc.tile_pool(name="consts", bufs=1) as consts,
            tc.tile_pool(name="adjp", bufs=4) as adjp,
            tc.tile_pool(name="up", bufs=2) as up,
            tc.tile_pool(name="qp", bufs=2) as qp,
            tc.tile_pool(name="tailp", bufs=2) as tailp,
        ):
            from concourse.masks import make_identity

            identity0 = consts.tile([128, 128], F32)
            make_identity(nc, identity0)
            identity = consts.tile([128, 128], F32)
            nc.vector.tensor_copy(identity, identity0)

            zcol = consts.tile([128, 1], F32)
            nc.vector.memset(zcol, 0.0)
            ones128 = consts.tile([128, 128], F16)
            nc.vector.memset(ones128, 1.0)
            # kD broadcast column (k*D0 replicated to all partitions)
            kdcol = consts.tile([128, 1], F32)
            kd_ap = kd_d[:, :]
            nc.gpsimd.dma_start(
                out=kdcol,
                in_=bass.AP(tensor=kd_ap.tensor, offset=0, ap=[[0, 128], [1, 1]]),
            )

            # a1/a2 broadcast across partitions (partition-step-0 DMA)
            a_ap = a_d[:, :]
            a1bc = consts.tile([128, dout], F32)
            nc.gpsimd.dma_start(
                out=a1bc,
                in_=bass.AP(tensor=a_ap.tensor, offset=0, ap=[[0, 128], [1, dout]]),
            )
            a2bc = consts.tile([128, dout], F32)
            nc.gpsimd.dma_start(
                out=a2bc,
                in_=bass.AP(
                    tensor=a_ap.tensor, offset=dout, ap=[[0, 128], [1, dout]]
                ),
            )

            # wx_k = [w_k | w_k@a1 | w_k@a2] in bf16, single DVE writer
            wx = []
            wxraw = []
            for k in range(kc):
                wxr = consts.tile([128, dout + 2], F32, name=f"wxr{k}")
                nc.sync.dma_start(
                    out=wxr[:, 0:dout], in_=w_d[k * 128 : (k + 1) * 128, :]
                )
                t1 = consts.tile([128, dout], F32, name=f"wa_t{k}")
                nc.vector.tensor_mul(t1, wxr[:, 0:dout], a1bc)
                nc.vector.reduce_sum(
                    wxr[:, dout : dout + 1], t1, axis=mybir.AxisListType.X
                )
                t2 = consts.tile([128, dout], F32, name=f"wb_t{k}")
                nc.vector.tensor_mul(t2, wxr[:, 0:dout], a2bc)
                nc.vector.reduce_sum(
                    wxr[:, dout + 1 : dout + 2], t2, axis=mybir.AxisListType.X
                )
                wxk = consts.tile([128, dout + 2], F16, name=f"wx{k}")
                nc.vector.tensor_copy(wxk, wxr)
                wx.append(wxk)
                wxraw.append(wxr)

            h_ext = consts.tile([128, ng, de], F32R)
            # memset can't write f32r; copy from a ones tile instead
            nc.vector.tensor_copy(
                h_ext[:, :, dout : dout + 1], ones128[:, 0:ng]
            )
            hz1bc = consts.tile([128, rows], F32)
            hz2cols = consts.tile([128, ng], F32)
            bias_cols = consts.tile([128, ng], F32)
            hpT = consts.tile([de, rows], F32)

            with (
                tc.tile_pool(name="xtp", bufs=1) as xtp,
                tc.tile_pool(name="pspre", bufs=2, space="PSUM") as pspre,
                tc.tile_pool(name="pshz", bufs=1, space="PSUM") as pshz,
            ):
                xts = []
                xchunk = 4096 if n >= 4096 else n
                for k in range(kc):
                    xtk = xtp.tile([128, n], F16, name=f"xt{k}")
                    for c0 in range(0, n, xchunk):
                        nc.sync.dma_start(
                            out=xtk[:, c0 : c0 + xchunk],
                            in_=xt_d[k * 128 : (k + 1) * 128, c0 : c0 + xchunk],
                        )
                    xts.append(xtk)
                xtos = []
                for k in range(kc):
                    xtok = xtp.tile([128, rows], F16, name=f"xto{k}")
                    nc.sync.dma_start(
                        out=xtok, in_=xto_d[k * 128 : (k + 1) * 128, :]
                    )
                    xtos.append(xtok)

                # h_ext (bf16) and hz2 columns per j-group
                for g0 in range(0, ng, grp):
                    ps = pspre.tile([128, grp, dout + 2], F32, name="ps_h")
                    for gi in range(grp):
                        g = g0 + gi
                        for k in range(kc):
                            nc.tensor.matmul(
                                ps[:, gi, :],
                                xts[k][:, g * 128 : (g + 1) * 128],
                                wx[k],
                                start=(k == 0),
                                stop=(k == kc - 1),
                            )
                    nc.vector.tensor_copy(
                        h_ext[:, g0 : g0 + grp, 0:dout], ps[:, :, 0:dout]
                    )
                    nc.vector.tensor_copy(
                        hz2cols[:, g0 : g0 + grp],
                        ps[:, :, dout + 1 : dout + 2],
                    )

                # hz1 broadcast row-block, prescaled by kpre
                hz_ps = pshz.tile([128, rows], F32)
                for k in range(kc):
                    wa1bc = consts.tile([128, 128], F16, name=f"wa1bc{k}")
                    nc.vector.tensor_scalar_mul(
                        wa1bc, ones128, wxraw[k][:, dout : dout + 1]
                    )
                    nwmax = 512
                    for n0 in range(0, rows, nwmax):
                        nw = min(nwmax, rows - n0)
                        nc.tensor.matmul(
                            hz_ps[:, n0 : n0 + nw],
                            wa1bc,
                            xtos[k][:, n0 : n0 + nw],
                            start=(k == 0),
                            stop=(k == kc - 1),
                        )
                nc.vector.tensor_scalar_mul(hz1bc, hz_ps, kpre)
                # bias_cols = kpre*hz2 + k*D0
                nc.vector.tensor_scalar(
                    bias_cols, hz2cols, kpre, kdcol[:, 0:1], OP.mult, OP.add
                )

            # main loop: stream encoded adjT megatiles (256 j x rows i)
            adjt_r = adjt_d[:, :].rearrange(
                "(m t p) i -> m p t i", t=2, p=128
            )
            with (
                tc.tile_pool(name="psacc", bufs=1, space="PSUM") as psacc,
                tc.tile_pool(name="pstail", bufs=2, space="PSUM") as pstail,
            ):
                acc = psacc.tile([de, rows], F32)
                for m in range(nm):
                    adjt_t = adjp.tile([128, 2 * rows], F16)
                    nc.sync.dma_start(
                        out=adjt_t.rearrange("p (t i) -> p t i", t=2),
                        in_=adjt_r[m],
                    )
                    u = up.tile([128, 2 * rows], F32)
                    for t in range(2):
                        g = 2 * m + t
                        nc.vector._custom_dve(
                            lmb,
                            out=u[:, t * rows : (t + 1) * rows],
                            in0=hz1bc,
                            in1=adjt_t[:, t * rows : (t + 1) * rows],
                            s0=bias_cols[:, g : g + 1],
                            s1=s1_slope,
                            imm2=-G,
                        )
                    q = qp.tile([128, 2 * rows], F32R)
                    nc.scalar.activation(q, u, AF.Exp, bias=zcol[:, 0:1])
                    for t in range(2):
                        g = 2 * m + t
                        nwmax = 512
                        for n0 in range(0, rows, nwmax):
                            nw = min(nwmax, rows - n0)
                            nc.tensor.matmul(
                                acc[:, n0 : n0 + nw],
                                h_ext[:, g, :],
                                q[:, t * rows + n0 : t * rows + n0 + nw],
                                start=(g == 0),
                                stop=(g == ng - 1),
                            )

                # tail: normalize + elu, back to i-major
                nc.vector.tensor_copy(hpT, acc)
                for cc in range(rows // 128):
                    tp = pstail.tile([128, de], F32)
                    nc.tensor.transpose(
                        tp,
                        hpT[:, cc * 128 : (cc + 1) * 128],
                        identity[0:de, 0:de],
                    )
                    recip = tailp.tile([128, 1], F32)
                    nc.vector.reciprocal(recip, tp[:, dout : dout + 1])
                    v = tailp.tile([128, dout], F32)
                    nc.vector.tensor_scalar_mul(v, tp[:, 0:dout], recip)
                    r = tailp.tile([128, dout], F32)
                    nc.scalar.activation(r, v, AF.Relu, bias=zcol[:, 0:1])
                    q2 = tailp.tile([128, dout], F32)
                    nc.scalar.activation(q2, v, AF.Exp, bias=zcol[:, 0:1])
                    m2 = tailp.tile([128, dout], F32)
                    nc.vector.tensor_scalar(m2, q2, 1.0, -1.0, OP.min, OP.add)
                    ysb = tailp.tile([128, dout], F32)
                    nc.vector.tensor_add(ysb, r, m2)
                    nc.sync.dma_start(
                        out=y_d[cc * 128 : (cc + 1) * 128, :], in_=ysb
                    )
    nc.compile()
    return nc


def _run(x, adj, w, a, a_coeff, b_coeff, c_coeff, d_coeff):
    global LAST_RESULTS, LAST_NC
    n, din = x.shape
    dout = w.shape[1]
    assert adj.shape == (n, n) and a.shape == (2 * dout, 1)
    rows = n // NCORES

    A = float(np.asarray(a_coeff).reshape(-1)[0])
    B = float(np.asarray(b_coeff).reshape(-1)[0])
    C = float(np.asarray(c_coeff).reshape(-1)[0])
    D0 = float(np.asarray(d_coeff).reshape(-1)[0])
    L1 = _leaky(A + B)

    x = np.ascontiguousarray(x, dtype=np.float32)
    adj = np.asarray(adj, dtype=np.float32)
    w = np.ascontiguousarray(w, dtype=np.float32)
    a = np.ascontiguousarray(a, dtype=np.float32)

    # host-side stability shift G >= max logit (from h extremes only)
    h = x @ w
    hz1 = h @ a[:dout, 0]
    hz2 = h @ a[dout:, 0]
    cand = []
    for u in (hz1.min(), hz1.max()):
        for v in (hz2.min(), hz2.max()):
            cand.append(L1 * _leaky(C * (float(u) + float(v)) + D0))
    G = float(max(cand))

    # leaky-scale trick (positive homogeneity of leaky)
    if L1 >= 0.0:
        kk, s1_slope = L1, SLOPE
    else:
        kk, s1_slope = SLOPE * L1, 1.0 / SLOPE
    kpre = kk * C

    nc = _build(n, din, dout, rows, kpre, s1_slope, G)
    LAST_NC = nc

    xt_b = np.ascontiguousarray(x.T).astype(np.float16)
    kd = np.full((1, 1), kk * D0, dtype=np.float32)
    in_maps = []
    for c in range(NCORES):
        sl = slice(c * rows, (c + 1) * rows)
        adjt_enc = ((adj[sl, :].T - 1.0) * BIG).astype(np.float16)
        in_maps.append(
            {
                "adjt": np.ascontiguousarray(adjt_enc),
                "xt": xt_b,
                "xt_own": np.ascontiguousarray(xt_b[:, sl]),
                "w": w,
                "a": a,
                "kd": kd,
            }
        )

    res = run_bass_kernel_spmd(
        nc, in_maps, core_ids=list(range(NCORES)), trace=TRACE
    )
    LAST_RESULTS = res
    return np.concatenate([r["y"] for r in res.results], axis=0)


def kernel(x, adj, w, a, a_coeff, b_coeff, c_coeff, d_coeff):
    return _run(x, adj, w, a, a_coeff, b_coeff, c_coeff, d_coeff)


# revision 22
# speedup vs baseline: 1.7420x; 1.0042x over previous
"""DGAT attention head on 8 trn2 NeuronCores.

Sharding: row-wise over query nodes (core c owns rows [c*R, (c+1)*R)).
Each core receives its adj slice pre-transposed and mask-encoded
(host-side layout choice): adjt_enc = (adj^T - 1) * BIG in bf16
(exact: adj is binary), so masking becomes an additive logit bias.

Math (exact for binary adj):
  h   = x @ w;  hz1 = x @ (w @ a[:D]);  hz2 = x @ (w @ a[D:])
  z   = C*(hz1[i] + hz2[j]) + D0;  L1 = leaky(A+B)   (leaky slope 0.2)
  row-softmax of masked logits  ==  normalize(exp(L1*leaky(z) - G + BIG*(adj-1)))
  out = elu((p @ h) / (p @ 1))
G is a host-derived bound on max logit (from hz1/hz2 extremes).

Device pipeline per 256-j megatile (j on partitions, i free):
  DMA 512KB bf16 adjt_enc
  -> custom DVE op: u = leakyscaled(Src0 + bias_col) + adjt_enc - G   (1 pass)
  -> ACT Exp: q = exp(u), bf16 out                                    (1 pass)
  -> PE: psum[65, R] += [h|1]^T-group @ q-half (bf16, N=1024)
Tail: psum -> sbuf, PE transposes [65,128]->[128,65], reciprocal of the
sum column, ELU = relu(v) + (min(exp(v),1) - 1).

The leaky-scale trick: for L1>=0, u_leak = select(w>=0, w, 0.2w) with
w = L1*z (positive homogeneity); for L1<0, w = 0.2*L1*z and the false
branch multiplies by 1/0.2.
"""

import numpy as np
import ml_dtypes

import concourse.bass as bass
import concourse.bacc as bacc
import concourse.mybir as mybir
import concourse.dve_ops as dve_ops
from concourse.dve_spec import Spec, Src0, Src1, C0, C1, C2, Zero, select
from concourse.tile import TileContext
from concourse.bass_utils import run_bass_kernel_spmd

F32 = mybir.dt.float32
F16 = mybir.dt.float16
F32R = mybir.dt.float32r
F8 = mybir.dt.float8e5
AF = mybir.ActivationFunctionType
OP = mybir.AluOpType

NCORES = 8
SLOPE = 0.2    # leakyrelu negative slope (fixed in the reference)
BIG = 16384.0  # additive mask magnitude (exact in fp16; exp(-BIG) == 0)

TRACE = False
LAST_RESULTS = None
LAST_NC = None


def _leaky(z):
    return z if z >= 0.0 else SLOPE * z


def _register_leaky_mask_op():
    name = "LEAKY_MASK_BIAS_ANT"
    for op in dve_ops.OPS:
        if op.name == name:
            return op
    w = Src0 + C0
    spec = Spec(
        body=select(w >= Zero, w, w * C1) + Src1 + C2,
        reference=lambda in0, in1, s0, s1, imm2: (
            np.where(in0 + s0 >= 0, in0 + s0, (in0 + s0) * s1) + in1 + imm2
        ).astype(np.float32),
    )
    op = dve_ops.DveOp(name, spec, subdim=False, uops_sha={})
    dve_ops.OPS.append(op)
    dve_ops.CUSTOM_DVE_SPECS[name] = spec
    dve_ops._SUB_OPCODE_FOR_NAME[name] = (
        dve_ops._CUSTOM_DVE_ROW_BASE + len(dve_ops.OPS) - 1
    )
    assert dve_ops._SUB_OPCODE_FOR_NAME[name] < 0x20
    for ver in ("v3",):
        try:
            op.compile(ver)
        except ValueError as e:
            msg = str(e)
            key = 'uops_sha["%s"]="' % ver
            i = msg.index(key) + len(key)
            op.uops_sha[ver] = msg[i : msg.index('"', i)]
        dve_ops._COMPILE_CACHE.pop((name, ver), None)
        op.compile(ver)
    return op


def _build(n, din, dout, rows, kpre, s1_slope, G):
    """Build the SPMD Bass program (identical on all cores).

    kpre: scale applied to hz1/hz2 logit halves (= k*C with k = L1 or
    SLOPE*L1); the per-partition bias col is kpre*hz2 + kD (kD folded on
    device); s1_slope: false-branch slope of the select (0.2 or 5.0).
    """
    assert n % 256 == 0 and rows % 128 == 0 and din % 128 == 0
    ng = n // 128
    mt = 4 if n % 512 == 0 else 2
    nm = n // (128 * mt)
    kc = din // 128
    grp = 4
    assert ng % grp == 0
    de = dout + 1
    lmb = _register_leaky_mask_op()

    nc = bacc.Bacc("TRN2", target_bir_lowering=False)
    adjt_d = nc.dram_tensor("adjt", [n, rows], F8, kind="ExternalInput")
    xt_d = nc.dram_tensor("xt", [din, n], F16, kind="ExternalInput")
    xto_d = nc.dram_tensor("xt_own", [din, rows], F16, kind="ExternalInput")
    w_d = nc.dram_tensor("w", [din, dout], F32, kind="ExternalInput")
    a_d = nc.dram_tensor("a", [2 * dout, 1], F32, kind="ExternalInput")
    kd_d = nc.dram_tensor("kd", [1, 1], F32, kind="ExternalInput")
    y_d = nc.dram_tensor("y", [rows, dout], F32, kind="ExternalOutput")

    with TileContext(nc) as tc:
        with (
            tc.tile_pool(name="consts", bufs=1) as consts,
            tc.tile_pool(name="adjp", bufs=4) as adjp,
            tc.tile_pool(name="up", bufs=2) as up,
            tc.tile_pool(name="qp", bufs=2) as qp,
            tc.tile_pool(name="tailp", bufs=2) as tailp,
        ):
            from concourse.masks import make_identity

            identity0 = consts.tile([128, 128], F32)
            make_identity(nc, identity0)
            identity = consts.tile([128, 128], F32)
            nc.vector.tensor_copy(identity, identity0)

            zcol = consts.tile([128, 1], F32)
            nc.vector.memset(zcol, 0.0)
            ones128 = consts.tile([128, 128], F16)
            nc.vector.memset(ones128, 1.0)
            # kD broadcast column (k*D0 replicated to all partitions)
            kdcol = consts.tile([128, 1], F32)
            kd_ap = kd_d[:, :]
            nc.gpsimd.dma_start(
                out=kdcol,
                in_=bass.AP(tensor=kd_ap.tensor, offset=0, ap=[[0, 128], [1, 1]]),
            )

            # a1/a2 broadcast across partitions (partition-step-0 DMA)
            a_ap = a_d[:, :]
            a1bc = consts.tile([128, dout], F32)
            nc.gpsimd.dma_start(
                out=a1bc,
                in_=bass.AP(tensor=a_ap.tensor, offset=0, ap=[[0, 128], [1, dout]]),
            )
            a2bc = consts.tile([128, dout], F32)
            nc.gpsimd.dma_start(
                out=a2bc,
                in_=bass.AP(
                    tensor=a_ap.tensor, offset=dout, ap=[[0, 128], [1, dout]]
                ),
            )

            # wx_k = [w_k | w_k@a1 | w_k@a2] in bf16, single DVE writer
            wx = []
            wxraw = []
            for k in range(kc):
                wxr = consts.tile([128, dout + 2], F32, name=f"wxr{k}")
                nc.sync.dma_start(
                    out=wxr[:, 0:dout], in_=w_d[k * 128 : (k + 1) * 128, :]
                )
                t1 = consts.tile([128, dout], F32, name=f"wa_t{k}")
                nc.vector.tensor_mul(t1, wxr[:, 0:dout], a1bc)
                nc.vector.reduce_sum(
                    wxr[:, dout : dout + 1], t1, axis=mybir.AxisListType.X
                )
                t2 = consts.tile([128, dout], F32, name=f"wb_t{k}")
                nc.vector.tensor_mul(t2, wxr[:, 0:dout], a2bc)
                nc.vector.reduce_sum(
                    wxr[:, dout + 1 : dout + 2], t2, axis=mybir.AxisListType.X
                )
                wxk = consts.tile([128, dout + 2], F16, name=f"wx{k}")
                nc.vector.tensor_copy(wxk, wxr)
                wx.append(wxk)
                wxraw.append(wxr)

            h_ext = consts.tile([128, ng, de], F32R)
            # memset can't write f32r; copy from a ones tile instead
            nc.vector.tensor_copy(
                h_ext[:, :, dout : dout + 1], ones128[:, 0:ng]
            )
            hz1bc = consts.tile([128, rows], F32)
            hz2cols = consts.tile([128, ng], F32)
            bias_cols = consts.tile([128, ng], F32)
            hpT = consts.tile([de, rows], F32)

            with (
                tc.tile_pool(name="xtp", bufs=1) as xtp,
                tc.tile_pool(name="pspre", bufs=2, space="PSUM") as pspre,
                tc.tile_pool(name="pshz", bufs=1, space="PSUM") as pshz,
            ):
                xts = []
                xchunk = 2048 if n >= 2048 else n
                for k in range(kc):
                    xtk = xtp.tile([128, n], F16, name=f"xt{k}")
                    for c0 in range(0, n, xchunk):
                        nc.sync.dma_start(
                            out=xtk[:, c0 : c0 + xchunk],
                            in_=xt_d[k * 128 : (k + 1) * 128, c0 : c0 + xchunk],
                        )
                    xts.append(xtk)
                xtos = []
                for k in range(kc):
                    xtok = xtp.tile([128, rows], F16, name=f"xto{k}")
                    nc.sync.dma_start(
                        out=xtok, in_=xto_d[k * 128 : (k + 1) * 128, :]
                    )
                    xtos.append(xtok)

                # h_ext (bf16) and hz2 columns per j-group
                for g0 in range(0, ng, grp):
                    ps = pspre.tile([128, grp, dout + 2], F32, name="ps_h")
                    for gi in range(grp):
                        g = g0 + gi
                        for k in range(kc):
                            nc.tensor.matmul(
                                ps[:, gi, :],
                                xts[k][:, g * 128 : (g + 1) * 128],
                                wx[k],
                                start=(k == 0),
                                stop=(k == kc - 1),
                            )
                    nc.vector.tensor_copy(
                        h_ext[:, g0 : g0 + grp, 0:dout], ps[:, :, 0:dout]
                    )
                    nc.vector.tensor_copy(
                        hz2cols[:, g0 : g0 + grp],
                        ps[:, :, dout + 1 : dout + 2],
                    )

                # hz1 broadcast row-block, prescaled by kpre
                hz_ps = pshz.tile([128, rows], F32)
                for k in range(kc):
                    wa1bc = consts.tile([128, 128], F16, name=f"wa1bc{k}")
                    nc.vector.tensor_scalar_mul(
                        wa1bc, ones128, wxraw[k][:, dout : dout + 1]
                    )
                    nwmax = 512
                    for n0 in range(0, rows, nwmax):
                        nw = min(nwmax, rows - n0)
                        nc.tensor.matmul(
                            hz_ps[:, n0 : n0 + nw],
                            wa1bc,
                            xtos[k][:, n0 : n0 + nw],
                            start=(k == 0),
                            stop=(k == kc - 1),
                        )
                nc.vector.tensor_scalar_mul(hz1bc, hz_ps, kpre)
                # bias_cols = kpre*hz2 + k*D0
                nc.vector.tensor_scalar(
                    bias_cols, hz2cols, kpre, kdcol[:, 0:1], OP.mult, OP.add
                )

            # main loop: stream encoded adjT megatiles (256 j x rows i)
            adjt_r = adjt_d[:, :].rearrange(
                "(m t p) i -> m p t i", t=mt, p=128
            )
            with (
                tc.tile_pool(name="psacc", bufs=1, space="PSUM") as psacc,
                tc.tile_pool(name="pstail", bufs=2, space="PSUM") as pstail,
            ):
                acc = psacc.tile([de, rows], F32)
                for m in range(nm):
                    adjt_t = adjp.tile([128, mt * rows], F8)
                    nc.sync.dma_start(
                        out=adjt_t.rearrange("p (t i) -> p t i", t=mt),
                        in_=adjt_r[m],
                    )
                    u = up.tile([128, mt * rows], F32)
                    for t in range(mt):
                        g = mt * m + t
                        nc.vector._custom_dve(
                            lmb,
                            out=u[:, t * rows : (t + 1) * rows],
                            in0=hz1bc,
                            in1=adjt_t[:, t * rows : (t + 1) * rows],
                            s0=bias_cols[:, g : g + 1],
                            s1=s1_slope,
                            imm2=-G,
                        )
                    q = qp.tile([128, mt * rows], F32R)
                    nc.scalar.activation(q, u, AF.Exp, bias=zcol[:, 0:1])
                    for t in range(mt):
                        g = mt * m + t
                        nwmax = 512
                        for n0 in range(0, rows, nwmax):
                            nw = min(nwmax, rows - n0)
                            nc.tensor.matmul(
                                acc[:, n0 : n0 + nw],
                                h_ext[:, g, :],
                                q[:, t * rows + n0 : t * rows + n0 + nw],
                                start=(g == 0),
                                stop=(g == ng - 1),
                            )

                # tail: normalize + elu, back to i-major
                nc.vector.tensor_copy(hpT, acc)
                for cc in range(rows // 128):
                    tp = pstail.tile([128, de], F32)
                    nc.tensor.transpose(
                        tp,
                        hpT[:, cc * 128 : (cc + 1) * 128],
                        identity[0:de, 0:de],
                    )
                    recip = tailp.tile([128, 1], F32)
                    nc.vector.reciprocal(recip, tp[:, dout : dout + 1])
                    v = tailp.tile([128, dout], F32)
                    nc.vector.tensor_scalar_mul(v, tp[:, 0:dout], recip)
                    r = tailp.tile([128, dout], F32)
                    nc.scalar.activation(r, v, AF.Relu, bias=zcol[:, 0:1])
                    q2 = tailp.tile([128, dout], F32)
                    nc.scalar.activation(q2, v, AF.Exp, bias=zcol[:, 0:1])
                    m2 = tailp.tile([128, dout], F32)
                    nc.vector.tensor_scalar(m2, q2, 1.0, -1.0, OP.min, OP.add)
                    ysb = tailp.tile([128, dout], F32)
                    nc.vector.tensor_add(ysb, r, m2)
                    nc.sync.dma_start(
                        out=y_d[cc * 128 : (cc + 1) * 128, :], in_=ysb
                    )
    nc.compile()
    return nc


def _run(x, adj, w, a, a_coeff, b_coeff, c_coeff, d_coeff):
    global LAST_RESULTS, LAST_NC
    n, din = x.shape
    dout = w.shape[1]
    assert adj.shape == (n, n) and a.shape == (2 * dout, 1)
    rows = n // NCORES

    A = float(np.asarray(a_coeff).reshape(-1)[0])
    B = float(np.asarray(b_coeff).reshape(-1)[0])
    C = float(np.asarray(c_coeff).reshape(-1)[0])
    D0 = float(np.asarray(d_coeff).reshape(-1)[0])
    L1 = _leaky(A + B)

    x = np.ascontiguousarray(x, dtype=np.float32)
    adj = np.asarray(adj, dtype=np.float32)
    w = np.ascontiguousarray(w, dtype=np.float32)
    a = np.ascontiguousarray(a, dtype=np.float32)

    # host-side stability shift G >= max logit (from h extremes only)
    h = x @ w
    hz1 = h @ a[:dout, 0]
    hz2 = h @ a[dout:, 0]
    cand = []
    for u in (hz1.min(), hz1.max()):
        for v in (hz2.min(), hz2.max()):
            cand.append(L1 * _leaky(C * (float(u) + float(v)) + D0))
    G = float(max(cand))

    # leaky-scale trick (positive homogeneity of leaky)
    if L1 >= 0.0:
        kk, s1_slope = L1, SLOPE
    else:
        kk, s1_slope = SLOPE * L1, 1.0 / SLOPE
    kpre = kk * C

    nc = _build(n, din, dout, rows, kpre, s1_slope, G)
    LAST_NC = nc

    xt_b = np.ascontiguousarray(x.T).astype(np.float16)
    kd = np.full((1, 1), kk * D0, dtype=np.float32)
    in_maps = []
    for c in range(NCORES):
        sl = slice(c * rows, (c + 1) * rows)
        adjt_enc = ((adj[sl, :].T - 1.0) * BIG).astype(ml_dtypes.float8_e5m2)
        in_maps.append(
            {
                "adjt": np.ascontiguousarray(adjt_enc),
                "xt": xt_b,
                "xt_own": np.ascontiguousarray(xt_b[:, sl]),
                "w": w,
                "a": a,
                "kd": kd,
            }
        )

    res = run_bass_kernel_spmd(
        nc, in_maps, core_ids=list(range(NCORES)), trace=TRACE
    )
    LAST_RESULTS = res
    return np.concatenate([r["y"] for r in res.results], axis=0)


def kernel(x, adj, w, a, a_coeff, b_coeff, c_coeff, d_coeff):
    return _run(x, adj, w, a, a_coeff, b_coeff, c_coeff, d_coeff)


# revision 23
# speedup vs baseline: 1.7523x; 1.0060x over previous
"""DGAT attention head on 8 trn2 NeuronCores.

Sharding: row-wise over query nodes (core c owns rows [c*R, (c+1)*R)).
Each core receives its adj slice pre-transposed and mask-encoded
(host-side layout choice): adjt_enc = (adj^T - 1) * BIG in bf16
(exact: adj is binary), so masking becomes an additive logit bias.

Math (exact for binary adj):
  h   = x @ w;  hz1 = x @ (w @ a[:D]);  hz2 = x @ (w @ a[D:])
  z   = C*(hz1[i] + hz2[j]) + D0;  L1 = leaky(A+B)   (leaky slope 0.2)
  row-softmax of masked logits  ==  normalize(exp(L1*leaky(z) - G + BIG*(adj-1)))
  out = elu((p @ h) / (p @ 1))
G is a host-derived bound on max logit (from hz1/hz2 extremes).

Device pipeline per 256-j megatile (j on partitions, i free):
  DMA 512KB bf16 adjt_enc
  -> custom DVE op: u = leakyscaled(Src0 + bias_col) + adjt_enc - G   (1 pass)
  -> ACT Exp: q = exp(u), bf16 out                                    (1 pass)
  -> PE: psum[65, R] += [h|1]^T-group @ q-half (bf16, N=1024)
Tail: psum -> sbuf, PE transposes [65,128]->[128,65], reciprocal of the
sum column, ELU = relu(v) + (min(exp(v),1) - 1).

The leaky-scale trick: for L1>=0, u_leak = select(w>=0, w, 0.2w) with
w = L1*z (positive homogeneity); for L1<0, w = 0.2*L1*z and the false
branch multiplies by 1/0.2.
"""

import numpy as np
import ml_dtypes

import concourse.bass as bass
import concourse.bacc as bacc
import concourse.mybir as mybir
import concourse.dve_ops as dve_ops
from concourse.dve_spec import Spec, Src0, Src1, C0, C1, C2, Zero, select
from concourse.tile import TileContext
from concourse.bass_utils import run_bass_kernel_spmd

F32 = mybir.dt.float32
F16 = mybir.dt.float16
F32R = mybir.dt.float32r
F8 = mybir.dt.float8e5
AF = mybir.ActivationFunctionType
OP = mybir.AluOpType

NCORES = 8
SLOPE = 0.2    # leakyrelu negative slope (fixed in the reference)
BIG = 16384.0  # additive mask magnitude (exact in fp16; exp(-BIG) == 0)

TRACE = False
LAST_RESULTS = None
LAST_NC = None


def _leaky(z):
    return z if z >= 0.0 else SLOPE * z


def _register_leaky_mask_op():
    name = "LEAKY_MASK_BIAS_ANT"
    for op in dve_ops.OPS:
        if op.name == name:
            return op
    w = Src0 + C0
    spec = Spec(
        body=select(w >= Zero, w, w * C1) + Src1 + C2,
        reference=lambda in0, in1, s0, s1, imm2: (
            np.where(in0 + s0 >= 0, in0 + s0, (in0 + s0) * s1) + in1 + imm2
        ).astype(np.float32),
    )
    op = dve_ops.DveOp(name, spec, subdim=False, uops_sha={})
    dve_ops.OPS.append(op)
    dve_ops.CUSTOM_DVE_SPECS[name] = spec
    dve_ops._SUB_OPCODE_FOR_NAME[name] = (
        dve_ops._CUSTOM_DVE_ROW_BASE + len(dve_ops.OPS) - 1
    )
    assert dve_ops._SUB_OPCODE_FOR_NAME[name] < 0x20
    for ver in ("v3",):
        try:
            op.compile(ver)
        except ValueError as e:
            msg = str(e)
            key = 'uops_sha["%s"]="' % ver
            i = msg.index(key) + len(key)
            op.uops_sha[ver] = msg[i : msg.index('"', i)]
        dve_ops._COMPILE_CACHE.pop((name, ver), None)
        op.compile(ver)
    return op


def _build(n, din, dout, rows, kpre, s1_slope, G):
    """Build the SPMD Bass program (identical on all cores).

    kpre: scale applied to hz1/hz2 logit halves (= k*C with k = L1 or
    SLOPE*L1); the per-partition bias col is kpre*hz2 + kD (kD folded on
    device); s1_slope: false-branch slope of the select (0.2 or 5.0).
    """
    assert n % 256 == 0 and rows % 128 == 0 and din % 128 == 0
    ng = n // 128
    mt = 4 if n % 512 == 0 else 2
    nm = n // (128 * mt)
    kc = din // 128
    grp = 4
    assert ng % grp == 0
    de = dout + 1
    lmb = _register_leaky_mask_op()

    nc = bacc.Bacc("TRN2", target_bir_lowering=False)
    adjt_d = nc.dram_tensor("adjt", [n, rows], F8, kind="ExternalInput")
    xt_d = nc.dram_tensor("xt", [din, n], F16, kind="ExternalInput")
    xto_d = nc.dram_tensor("xt_own", [din, rows], F16, kind="ExternalInput")
    w_d = nc.dram_tensor("w", [din, dout], F32, kind="ExternalInput")
    a_d = nc.dram_tensor("a", [2 * dout, 1], F32, kind="ExternalInput")
    kd_d = nc.dram_tensor("kd", [1, 1], F32, kind="ExternalInput")
    y_d = nc.dram_tensor("y", [rows, dout], F32, kind="ExternalOutput")

    with TileContext(nc) as tc:
        with (
            tc.tile_pool(name="consts", bufs=1) as consts,
            tc.tile_pool(name="adjp", bufs=6) as adjp,
            tc.tile_pool(name="up", bufs=2) as up,
            tc.tile_pool(name="qp", bufs=2) as qp,
            tc.tile_pool(name="tailp", bufs=2) as tailp,
        ):
            from concourse.masks import make_identity

            identity0 = consts.tile([128, 128], F32)
            make_identity(nc, identity0)
            identity = consts.tile([128, 128], F32)
            nc.vector.tensor_copy(identity, identity0)

            zcol = consts.tile([128, 1], F32)
            nc.vector.memset(zcol, 0.0)
            ones128 = consts.tile([128, 128], F16)
            nc.vector.memset(ones128, 1.0)
            # kD broadcast column (k*D0 replicated to all partitions)
            kdcol = consts.tile([128, 1], F32)
            kd_ap = kd_d[:, :]
            nc.gpsimd.dma_start(
                out=kdcol,
                in_=bass.AP(tensor=kd_ap.tensor, offset=0, ap=[[0, 128], [1, 1]]),
            )

            # a1/a2 broadcast across partitions (partition-step-0 DMA)
            a_ap = a_d[:, :]
            a1bc = consts.tile([128, dout], F32)
            nc.gpsimd.dma_start(
                out=a1bc,
                in_=bass.AP(tensor=a_ap.tensor, offset=0, ap=[[0, 128], [1, dout]]),
            )
            a2bc = consts.tile([128, dout], F32)
            nc.gpsimd.dma_start(
                out=a2bc,
                in_=bass.AP(
                    tensor=a_ap.tensor, offset=dout, ap=[[0, 128], [1, dout]]
                ),
            )

            # wx_k = [w_k | w_k@a1 | w_k@a2] in bf16, single DVE writer
            wx = []
            wxraw = []
            for k in range(kc):
                wxr = consts.tile([128, dout + 2], F32, name=f"wxr{k}")
                nc.sync.dma_start(
                    out=wxr[:, 0:dout], in_=w_d[k * 128 : (k + 1) * 128, :]
                )
                t1 = consts.tile([128, dout], F32, name=f"wa_t{k}")
                nc.vector.tensor_mul(t1, wxr[:, 0:dout], a1bc)
                nc.vector.reduce_sum(
                    wxr[:, dout : dout + 1], t1, axis=mybir.AxisListType.X
                )
                t2 = consts.tile([128, dout], F32, name=f"wb_t{k}")
                nc.vector.tensor_mul(t2, wxr[:, 0:dout], a2bc)
                nc.vector.reduce_sum(
                    wxr[:, dout + 1 : dout + 2], t2, axis=mybir.AxisListType.X
                )
                wxk = consts.tile([128, dout + 2], F16, name=f"wx{k}")
                nc.vector.tensor_copy(wxk, wxr)
                wx.append(wxk)
                wxraw.append(wxr)

            h_ext = consts.tile([128, ng, de], F32R)
            # memset can't write f32r; copy from a ones tile instead
            nc.vector.tensor_copy(
                h_ext[:, :, dout : dout + 1], ones128[:, 0:ng]
            )
            hz1bc = consts.tile([128, rows], F32)
            hz2cols = consts.tile([128, ng], F32)
            bias_cols = consts.tile([128, ng], F32)
            hpT = consts.tile([de, rows], F32)

            with (
                tc.tile_pool(name="xtp", bufs=1) as xtp,
                tc.tile_pool(name="pshz", bufs=1, space="PSUM") as pshz,
                tc.tile_pool(name="pspre", bufs=2, space="PSUM") as pspre,
            ):
                # own-x columns + hz1 broadcast first: this unblocks the
                # main-loop custom ops as early as possible
                xtos = []
                for k in range(kc):
                    xtok = xtp.tile([128, rows], F16, name=f"xto{k}")
                    nc.sync.dma_start(
                        out=xtok, in_=xto_d[k * 128 : (k + 1) * 128, :]
                    )
                    xtos.append(xtok)
                hz_ps = pshz.tile([128, rows], F32)
                for k in range(kc):
                    wa1bc = consts.tile([128, 128], F16, name=f"wa1bc{k}")
                    nc.vector.tensor_scalar_mul(
                        wa1bc, ones128, wxraw[k][:, dout : dout + 1]
                    )
                    nwmax = 512
                    for n0 in range(0, rows, nwmax):
                        nw = min(nwmax, rows - n0)
                        nc.tensor.matmul(
                            hz_ps[:, n0 : n0 + nw],
                            wa1bc,
                            xtos[k][:, n0 : n0 + nw],
                            start=(k == 0),
                            stop=(k == kc - 1),
                        )
                nc.vector.tensor_scalar_mul(hz1bc, hz_ps, kpre)

                xts = []
                xchunk = 2048 if n >= 2048 else n
                for k in range(kc):
                    xtk = xtp.tile([128, n], F16, name=f"xt{k}")
                    for c0 in range(0, n, xchunk):
                        nc.sync.dma_start(
                            out=xtk[:, c0 : c0 + xchunk],
                            in_=xt_d[k * 128 : (k + 1) * 128, c0 : c0 + xchunk],
                        )
                    xts.append(xtk)

                # h_ext (f32r), hz2 and bias columns per j-group, in the
                # order the main loop consumes them
                for g0 in range(0, ng, grp):
                    ps = pspre.tile([128, grp, dout + 2], F32, name="ps_h")
                    for gi in range(grp):
                        g = g0 + gi
                        for k in range(kc):
                            nc.tensor.matmul(
                                ps[:, gi, :],
                                xts[k][:, g * 128 : (g + 1) * 128],
                                wx[k],
                                start=(k == 0),
                                stop=(k == kc - 1),
                            )
                    nc.vector.tensor_copy(
                        h_ext[:, g0 : g0 + grp, 0:dout], ps[:, :, 0:dout]
                    )
                    nc.vector.tensor_copy(
                        hz2cols[:, g0 : g0 + grp],
                        ps[:, :, dout + 1 : dout + 2],
                    )
                    nc.vector.tensor_scalar(
                        bias_cols[:, g0 : g0 + grp],
                        hz2cols[:, g0 : g0 + grp],
                        kpre,
                        kdcol[:, 0:1],
                        OP.mult,
                        OP.add,
                    )

            # main loop: stream encoded adjT megatiles (256 j x rows i)
            adjt_r = adjt_d[:, :].rearrange(
                "(m t p) i -> m p t i", t=mt, p=128
            )
            with (
                tc.tile_pool(name="psacc", bufs=1, space="PSUM") as psacc,
                tc.tile_pool(name="pstail", bufs=2, space="PSUM") as pstail,
            ):
                acc = psacc.tile([de, rows], F32)
                for m in range(nm):
                    adjt_t = adjp.tile([128, mt * rows], F8)
                    nc.sync.dma_start(
                        out=adjt_t.rearrange("p (t i) -> p t i", t=mt),
                        in_=adjt_r[m],
                    )
                    u = up.tile([128, mt * rows], F32)
                    for t in range(mt):
                        g = mt * m + t
                        nc.vector._custom_dve(
                            lmb,
                            out=u[:, t * rows : (t + 1) * rows],
                            in0=hz1bc,
                            in1=adjt_t[:, t * rows : (t + 1) * rows],
                            s0=bias_cols[:, g : g + 1],
                            s1=s1_slope,
                            imm2=-G,
                        )
                    q = qp.tile([128, mt * rows], F32R)
                    nc.scalar.activation(q, u, AF.Exp, bias=zcol[:, 0:1])
                    for t in range(mt):
                        g = mt * m + t
                        nwmax = 512
                        for n0 in range(0, rows, nwmax):
                            nw = min(nwmax, rows - n0)
                            nc.tensor.matmul(
                                acc[:, n0 : n0 + nw],
                                h_ext[:, g, :],
                                q[:, t * rows + n0 : t * rows + n0 + nw],
                                start=(g == 0),
                                stop=(g == ng - 1),
                            )

                # tail: normalize + elu, back to i-major
                nc.vector.tensor_copy(hpT, acc)
                for cc in range(rows // 128):
                    tp = pstail.tile([128, de], F32)
                    nc.tensor.transpose(
                        tp,
                        hpT[:, cc * 128 : (cc + 1) * 128],
                        identity[0:de, 0:de],
                    )
                    recip = tailp.tile([128, 1], F32)
                    nc.vector.reciprocal(recip, tp[:, dout : dout + 1])
                    v = tailp.tile([128, dout], F32)
                    nc.vector.tensor_scalar_mul(v, tp[:, 0:dout], recip)
                    r = tailp.tile([128, dout], F32)
                    nc.scalar.activation(r, v, AF.Relu, bias=zcol[:, 0:1])
                    q2 = tailp.tile([128, dout], F32)
                    nc.scalar.activation(q2, v, AF.Exp, bias=zcol[:, 0:1])
                    m2 = tailp.tile([128, dout], F32)
                    nc.vector.tensor_scalar(m2, q2, 1.0, -1.0, OP.min, OP.add)
                    ysb = tailp.tile([128, dout], F32)
                    nc.vector.tensor_add(ysb, r, m2)
                    nc.sync.dma_start(
                        out=y_d[cc * 128 : (cc + 1) * 128, :], in_=ysb
                    )
    nc.compile()
    return nc


def _run(x, adj, w, a, a_coeff, b_coeff, c_coeff, d_coeff):
    global LAST_RESULTS, LAST_NC
    n, din = x.shape
    dout = w.shape[1]
    assert adj.shape == (n, n) and a.shape == (2 * dout, 1)
    rows = n // NCORES

    A = float(np.asarray(a_coeff).reshape(-1)[0])
    B = float(np.asarray(b_coeff).reshape(-1)[0])
    C = float(np.asarray(c_coeff).reshape(-1)[0])
    D0 = float(np.asarray(d_coeff).reshape(-1)[0])
    L1 = _leaky(A + B)

    x = np.ascontiguousarray(x, dtype=np.float32)
    adj = np.asarray(adj, dtype=np.float32)
    w = np.ascontiguousarray(w, dtype=np.float32)
    a = np.ascontiguousarray(a, dtype=np.float32)

    # host-side stability shift G >= max logit (from h extremes only)
    h = x @ w
    hz1 = h @ a[:dout, 0]
    hz2 = h @ a[dout:, 0]
    cand = []
    for u in (hz1.min(), hz1.max()):
        for v in (hz2.min(), hz2.max()):
            cand.append(L1 * _leaky(C * (float(u) + float(v)) + D0))
    G = float(max(cand))

    # leaky-scale trick (positive homogeneity of leaky)
    if L1 >= 0.0:
        kk, s1_slope = L1, SLOPE
    else:
        kk, s1_slope = SLOPE * L1, 1.0 / SLOPE
    kpre = kk * C

    nc = _build(n, din, dout, rows, kpre, s1_slope, G)
    LAST_NC = nc

    xt_b = np.ascontiguousarray(x.T).astype(np.float16)
    kd = np.full((1, 1), kk * D0, dtype=np.float32)
    in_maps = []
    for c in range(NCORES):
        sl = slice(c * rows, (c + 1) * rows)
        adjt_enc = ((adj[sl, :].T - 1.0) * BIG).astype(ml_dtypes.float8_e5m2)
        in_maps.append(
            {
                "adjt": np.ascontiguousarray(adjt_enc),
                "xt": xt_b,
                "xt_own": np.ascontiguousarray(xt_b[:, sl]),
                "w": w,
                "a": a,
                "kd": kd,
            }
        )

    res = run_bass_kernel_spmd(
        nc, in_maps, core_ids=list(range(NCORES)), trace=TRACE
    )
    LAST_RESULTS = res
    return np.concatenate([r["y"] for r in res.results], axis=0)


def kernel(x, adj, w, a, a_coeff, b_coeff, c_coeff, d_coeff):
    return _run(x, adj, w, a, a_coeff, b_coeff, c_coeff, d_coeff)


# revision 24
# speedup vs baseline: 1.8278x; 1.0430x over previous
"""DGAT attention head on 8 trn2 NeuronCores.

Sharding: row-wise over query nodes (core c owns rows [c*R, (c+1)*R)).
Each core receives its adj slice pre-transposed and mask-encoded
(host-side layout choice): adjt_enc = (adj^T - 1) * BIG in bf16
(exact: adj is binary), so masking becomes an additive logit bias.

Math (exact for binary adj):
  h   = x @ w;  hz1 = x @ (w @ a[:D]);  hz2 = x @ (w @ a[D:])
  z   = C*(hz1[i] + hz2[j]) + D0;  L1 = leaky(A+B)   (leaky slope 0.2)
  row-softmax of masked logits  ==  normalize(exp(L1*leaky(z) - G + BIG*(adj-1)))
  out = elu((p @ h) / (p @ 1))
G is a host-derived bound on max logit (from hz1/hz2 extremes).

Device pipeline per 256-j megatile (j on partitions, i free):
  DMA 512KB bf16 adjt_enc
  -> custom DVE op: u = leakyscaled(Src0 + bias_col) + adjt_enc - G   (1 pass)
  -> ACT Exp: q = exp(u), bf16 out                                    (1 pass)
  -> PE: psum[65, R] += [h|1]^T-group @ q-half (bf16, N=1024)
Tail: psum -> sbuf, PE transposes [65,128]->[128,65], reciprocal of the
sum column, ELU = relu(v) + (min(exp(v),1) - 1).

The leaky-scale trick: for L1>=0, u_leak = select(w>=0, w, 0.2w) with
w = L1*z (positive homogeneity); for L1<0, w = 0.2*L1*z and the false
branch multiplies by 1/0.2.
"""

import numpy as np
import ml_dtypes

import concourse.bass as bass
import concourse.bacc as bacc
import concourse.mybir as mybir
import concourse.dve_ops as dve_ops
from concourse.dve_spec import Spec, Src0, Src1, C0, C1, C2, Zero, select
from concourse.tile import TileContext
from concourse.bass_utils import run_bass_kernel_spmd

F32 = mybir.dt.float32
F16 = mybir.dt.float16
F32R = mybir.dt.float32r
F8 = mybir.dt.float8e5
AF = mybir.ActivationFunctionType
OP = mybir.AluOpType

NCORES = 8
SLOPE = 0.2    # leakyrelu negative slope (fixed in the reference)
BIG = 16384.0  # additive mask magnitude (exact in fp16; exp(-BIG) == 0)

TRACE = False
LAST_RESULTS = None
LAST_NC = None


def _leaky(z):
    return z if z >= 0.0 else SLOPE * z


def _register_leaky_mask_op():
    name = "LEAKY_MASK_BIAS_ANT"
    for op in dve_ops.OPS:
        if op.name == name:
            return op
    w = Src0 + C0
    spec = Spec(
        body=select(w >= Zero, w, w * C1) + Src1 + C2,
        reference=lambda in0, in1, s0, s1, imm2: (
            np.where(in0 + s0 >= 0, in0 + s0, (in0 + s0) * s1) + in1 + imm2
        ).astype(np.float32),
    )
    op = dve_ops.DveOp(name, spec, subdim=False, uops_sha={})
    dve_ops.OPS.append(op)
    dve_ops.CUSTOM_DVE_SPECS[name] = spec
    dve_ops._SUB_OPCODE_FOR_NAME[name] = (
        dve_ops._CUSTOM_DVE_ROW_BASE + len(dve_ops.OPS) - 1
    )
    assert dve_ops._SUB_OPCODE_FOR_NAME[name] < 0x20
    for ver in ("v3",):
        try:
            op.compile(ver)
        except ValueError as e:
            msg = str(e)
            key = 'uops_sha["%s"]="' % ver
            i = msg.index(key) + len(key)
            op.uops_sha[ver] = msg[i : msg.index('"', i)]
        dve_ops._COMPILE_CACHE.pop((name, ver), None)
        op.compile(ver)
    return op


def _build(n, din, dout, rows, kpre, s1_slope, G):
    """Build the SPMD Bass program (identical on all cores).

    kpre: scale applied to hz1/hz2 logit halves (= k*C with k = L1 or
    SLOPE*L1); the per-partition bias col is kpre*hz2 + kD (kD folded on
    device); s1_slope: false-branch slope of the select (0.2 or 5.0).
    """
    assert n % 256 == 0 and rows % 128 == 0 and din % 128 == 0
    ng = n // 128
    mt = 4 if n % 512 == 0 else 2
    nm = n // (128 * mt)
    kc = din // 128
    grp = 4
    assert ng % grp == 0
    de = dout + 1
    lmb = _register_leaky_mask_op()

    nc = bacc.Bacc("TRN2", target_bir_lowering=False)
    adjt_d = nc.dram_tensor("adjt", [n, rows], F8, kind="ExternalInput")
    xt_d = nc.dram_tensor("xt", [din, n], F16, kind="ExternalInput")
    xto_d = nc.dram_tensor("xt_own", [din, rows], F16, kind="ExternalInput")
    w_d = nc.dram_tensor("w", [din, dout], F32, kind="ExternalInput")
    a_d = nc.dram_tensor("a", [2 * dout, 1], F32, kind="ExternalInput")
    kd_d = nc.dram_tensor("kd", [1, 1], F32, kind="ExternalInput")
    y_d = nc.dram_tensor("y", [rows, dout], F32, kind="ExternalOutput")

    with TileContext(nc) as tc:
        with (
            tc.tile_pool(name="consts", bufs=1) as consts,
            tc.tile_pool(name="adjp", bufs=6) as adjp,
            tc.tile_pool(name="up", bufs=2) as up,
            tc.tile_pool(name="qp", bufs=2) as qp,
            tc.tile_pool(name="tailp", bufs=2) as tailp,
        ):
            from concourse.masks import make_identity

            identity0 = consts.tile([128, 128], F32)
            make_identity(nc, identity0)
            identity = consts.tile([128, 128], F32)
            nc.vector.tensor_copy(identity, identity0)

            zcol = consts.tile([128, 1], F32)
            nc.vector.memset(zcol, 0.0)
            ones128 = consts.tile([128, 128], F16)
            nc.vector.memset(ones128, 1.0)
            # kD broadcast column (k*D0 replicated to all partitions)
            kdcol = consts.tile([128, 1], F32)
            kd_ap = kd_d[:, :]
            nc.gpsimd.dma_start(
                out=kdcol,
                in_=bass.AP(tensor=kd_ap.tensor, offset=0, ap=[[0, 128], [1, 1]]),
            )

            # a1/a2 broadcast across partitions (partition-step-0 DMA)
            a_ap = a_d[:, :]
            a1bc = consts.tile([128, dout], F32)
            nc.gpsimd.dma_start(
                out=a1bc,
                in_=bass.AP(tensor=a_ap.tensor, offset=0, ap=[[0, 128], [1, dout]]),
            )
            a2bc = consts.tile([128, dout], F32)
            nc.gpsimd.dma_start(
                out=a2bc,
                in_=bass.AP(
                    tensor=a_ap.tensor, offset=dout, ap=[[0, 128], [1, dout]]
                ),
            )

            # wx_k = [w_k | w_k@a1 | w_k@a2] in bf16, single DVE writer
            wx = []
            wxraw = []
            for k in range(kc):
                wxr = consts.tile([128, dout + 2], F32, name=f"wxr{k}")
                nc.sync.dma_start(
                    out=wxr[:, 0:dout], in_=w_d[k * 128 : (k + 1) * 128, :]
                )
                t1 = consts.tile([128, dout], F32, name=f"wa_t{k}")
                nc.vector.tensor_mul(t1, wxr[:, 0:dout], a1bc)
                nc.vector.reduce_sum(
                    wxr[:, dout : dout + 1], t1, axis=mybir.AxisListType.X
                )
                t2 = consts.tile([128, dout], F32, name=f"wb_t{k}")
                nc.vector.tensor_mul(t2, wxr[:, 0:dout], a2bc)
                nc.vector.reduce_sum(
                    wxr[:, dout + 1 : dout + 2], t2, axis=mybir.AxisListType.X
                )
                wxk = consts.tile([128, dout + 2], F16, name=f"wx{k}")
                nc.vector.tensor_copy(wxk, wxr)
                wx.append(wxk)
                wxraw.append(wxr)

            h_ext = consts.tile([128, ng, de], F32R)
            # memset can't write f32r; copy from a ones tile instead
            nc.vector.tensor_copy(
                h_ext[:, :, dout : dout + 1], ones128[:, 0:ng]
            )
            hz1bc = consts.tile([128, rows], F32)
            hz2cols = consts.tile([128, ng], F32)
            bias_cols = consts.tile([128, ng], F32)
            hpT = consts.tile([de, rows], F32)

            with (
                tc.tile_pool(name="xtp", bufs=1) as xtp,
                tc.tile_pool(name="pshz", bufs=1, space="PSUM") as pshz,
                tc.tile_pool(name="pspre", bufs=2, space="PSUM") as pspre,
            ):
                # own-x columns + hz1 broadcast first: this unblocks the
                # main-loop custom ops as early as possible
                xtos = []
                for k in range(kc):
                    xtok = xtp.tile([128, rows], F16, name=f"xto{k}")
                    nc.sync.dma_start(
                        out=xtok, in_=xto_d[k * 128 : (k + 1) * 128, :]
                    )
                    xtos.append(xtok)
                hz_ps = pshz.tile([128, rows], F32)
                for k in range(kc):
                    wa1bc = consts.tile([128, 128], F16, name=f"wa1bc{k}")
                    nc.vector.tensor_scalar_mul(
                        wa1bc, ones128, wxraw[k][:, dout : dout + 1]
                    )
                    nwmax = 512
                    for n0 in range(0, rows, nwmax):
                        nw = min(nwmax, rows - n0)
                        nc.tensor.matmul(
                            hz_ps[:, n0 : n0 + nw],
                            wa1bc,
                            xtos[k][:, n0 : n0 + nw],
                            start=(k == 0),
                            stop=(k == kc - 1),
                        )
                nc.vector.tensor_scalar_mul(hz1bc, hz_ps, kpre)

                xts = []
                xchunk = 2048 if n >= 2048 else n
                for k in range(kc):
                    xtk = xtp.tile([128, n], F16, name=f"xt{k}")
                    for c0 in range(0, n, xchunk):
                        nc.sync.dma_start(
                            out=xtk[:, c0 : c0 + xchunk],
                            in_=xt_d[k * 128 : (k + 1) * 128, c0 : c0 + xchunk],
                        )
                    xts.append(xtk)

                # h_ext (f32r), hz2 and bias columns per j-group, in the
                # order the main loop consumes them
                for g0 in range(0, ng, grp):
                    ps = pspre.tile([128, grp, dout + 2], F32, name="ps_h")
                    for gi in range(grp):
                        g = g0 + gi
                        for k in range(kc):
                            nc.tensor.matmul(
                                ps[:, gi, :],
                                xts[k][:, g * 128 : (g + 1) * 128],
                                wx[k],
                                start=(k == 0),
                                stop=(k == kc - 1),
                            )
                    nc.scalar.copy(
                        h_ext[:, g0 : g0 + grp, 0:dout], ps[:, :, 0:dout]
                    )
                    nc.scalar.copy(
                        hz2cols[:, g0 : g0 + grp],
                        ps[:, :, dout + 1 : dout + 2],
                    )
                    nc.vector.tensor_scalar(
                        bias_cols[:, g0 : g0 + grp],
                        hz2cols[:, g0 : g0 + grp],
                        kpre,
                        kdcol[:, 0:1],
                        OP.mult,
                        OP.add,
                    )

            # main loop: stream encoded adjT megatiles (256 j x rows i)
            adjt_r = adjt_d[:, :].rearrange(
                "(m t p) i -> m p t i", t=mt, p=128
            )
            with (
                tc.tile_pool(name="psacc", bufs=1, space="PSUM") as psacc,
                tc.tile_pool(name="pstail", bufs=2, space="PSUM") as pstail,
            ):
                acc = psacc.tile([de, rows], F32)
                for m in range(nm):
                    adjt_t = adjp.tile([128, mt * rows], F8)
                    nc.sync.dma_start(
                        out=adjt_t.rearrange("p (t i) -> p t i", t=mt),
                        in_=adjt_r[m],
                    )
                    u = up.tile([128, mt * rows], F32)
                    for t in range(mt):
                        g = mt * m + t
                        nc.vector._custom_dve(
                            lmb,
                            out=u[:, t * rows : (t + 1) * rows],
                            in0=hz1bc,
                            in1=adjt_t[:, t * rows : (t + 1) * rows],
                            s0=bias_cols[:, g : g + 1],
                            s1=s1_slope,
                            imm2=-G,
                        )
                    q = qp.tile([128, mt * rows], F32R)
                    nc.scalar.activation(q, u, AF.Exp, bias=zcol[:, 0:1])
                    for t in range(mt):
                        g = mt * m + t
                        nwmax = 512
                        for n0 in range(0, rows, nwmax):
                            nw = min(nwmax, rows - n0)
                            nc.tensor.matmul(
                                acc[:, n0 : n0 + nw],
                                h_ext[:, g, :],
                                q[:, t * rows + n0 : t * rows + n0 + nw],
                                start=(g == 0),
                                stop=(g == ng - 1),
                            )

                # tail: normalize + elu, back to i-major
                nc.scalar.copy(hpT, acc)
                for cc in range(rows // 128):
                    tp = pstail.tile([128, de], F32)
                    nc.tensor.transpose(
                        tp,
                        hpT[:, cc * 128 : (cc + 1) * 128],
                        identity[0:de, 0:de],
                    )
                    recip = tailp.tile([128, 1], F32)
                    nc.vector.reciprocal(recip, tp[:, dout : dout + 1])
                    v = tailp.tile([128, dout], F32)
                    nc.vector.tensor_scalar_mul(v, tp[:, 0:dout], recip)
                    r = tailp.tile([128, dout], F32)
                    nc.scalar.activation(r, v, AF.Relu, bias=zcol[:, 0:1])
                    q2 = tailp.tile([128, dout], F32)
                    nc.scalar.activation(q2, v, AF.Exp, bias=zcol[:, 0:1])
                    m2 = tailp.tile([128, dout], F32)
                    nc.vector.tensor_scalar(m2, q2, 1.0, -1.0, OP.min, OP.add)
                    ysb = tailp.tile([128, dout], F32)
                    nc.vector.tensor_add(ysb, r, m2)
                    nc.sync.dma_start(
                        out=y_d[cc * 128 : (cc + 1) * 128, :], in_=ysb
                    )
    nc.compile()
    return nc


def _run(x, adj, w, a, a_coeff, b_coeff, c_coeff, d_coeff):
    global LAST_RESULTS, LAST_NC
    n, din = x.shape
    dout = w.shape[1]
    assert adj.shape == (n, n) and a.shape == (2 * dout, 1)
    rows = n // NCORES

    A = float(np.asarray(a_coeff).reshape(-1)[0])
    B = float(np.asarray(b_coeff).reshape(-1)[0])
    C = float(np.asarray(c_coeff).reshape(-1)[0])
    D0 = float(np.asarray(d_coeff).reshape(-1)[0])
    L1 = _leaky(A + B)

    x = np.ascontiguousarray(x, dtype=np.float32)
    adj = np.asarray(adj, dtype=np.float32)
    w = np.ascontiguousarray(w, dtype=np.float32)
    a = np.ascontiguousarray(a, dtype=np.float32)

    # host-side stability shift G >= max logit (from h extremes only)
    h = x @ w
    hz1 = h @ a[:dout, 0]
    hz2 = h @ a[dout:, 0]
    cand = []
    for u in (hz1.min(), hz1.max()):
        for v in (hz2.min(), hz2.max()):
            cand.append(L1 * _leaky(C * (float(u) + float(v)) + D0))
    G = float(max(cand))

    # leaky-scale trick (positive homogeneity of leaky)
    if L1 >= 0.0:
        kk, s1_slope = L1, SLOPE
    else:
        kk, s1_slope = SLOPE * L1, 1.0 / SLOPE
    kpre = kk * C

    nc = _build(n, din, dout, rows, kpre, s1_slope, G)
    LAST_NC = nc

    xt_b = np.ascontiguousarray(x.T).astype(np.float16)
    kd = np.full((1, 1), kk * D0, dtype=np.float32)
    in_maps = []
    for c in range(NCORES):
        sl = slice(c * rows, (c + 1) * rows)
        adjt_enc = ((adj[sl, :].T - 1.0) * BIG).astype(ml_dtypes.float8_e5m2)
        in_maps.append(
            {
                "adjt": np.ascontiguousarray(adjt_enc),
                "xt": xt_b,
                "xt_own": np.ascontiguousarray(xt_b[:, sl]),
                "w": w,
                "a": a,
                "kd": kd,
            }
        )

    res = run_bass_kernel_spmd(
        nc, in_maps, core_ids=list(range(NCORES)), trace=TRACE
    )
    LAST_RESULTS = res
    return np.concatenate([r["y"] for r in res.results], axis=0)


def kernel(x, adj, w, a, a_coeff, b_coeff, c_coeff, d_coeff):
    return _run(x, adj, w, a, a_coeff, b_coeff, c_coeff, d_coeff)
